# revision 1
# baseline (speedup 1.0000x reference)
import sys

sys.path.insert(0, "/opt/trn_rl_repo")
import numpy as np
import ml_dtypes
import concourse.bacc as bacc
import concourse.mybir as mybir
from concourse.tile import TileContext
from concourse.bass_utils import run_bass_kernel_spmd
from concourse.masks import make_identity

dt = mybir.dt

P = 128
B, S, H, I = 2, 2048, 2048, 8192
NCORES = 8
T = (B * S) // NCORES          # 512 tokens per core
KT1 = H // P                   # 16 k-tiles for matmul1
CH1 = 256                      # i-chunk width for phase 1
NI = I // CH1                  # 32 i-chunks
KPC = CH1 // P                 # 2 k-tiles (of matmul2) per i-chunk
KT2 = I // P                   # 64 k-tiles for matmul2
NH = 4                         # h-chunks of 512 for phase 2
MT = T // P                    # 4 token tiles per core

AF = mybir.ActivationFunctionType
ALU = mybir.AluOpType

_built = None


def _build():
    nc = bacc.Bacc(None, target_bir_lowering=False)
    xT = nc.dram_tensor("xT", [H, T], dt.float32r, kind="ExternalInput")
    w1T = nc.dram_tensor("w1T", [H, I], dt.float32r, kind="ExternalInput")
    w2T = nc.dram_tensor("w2T", [I, H], dt.float16, kind="ExternalInput")
    y3 = nc.dram_tensor("y3", [T, H], dt.float32, kind="ExternalOutput")

    with TileContext(nc) as tc:
        with (
            tc.tile_pool(name="const", bufs=1) as constp,
            tc.tile_pool(name="xsb", bufs=1) as xp,
            tc.tile_pool(name="w1p", bufs=3) as w1p,
            tc.tile_pool(name="w2p", bufs=2) as w2p,
            tc.tile_pool(name="act", bufs=3) as actp,
            tc.tile_pool(name="y2stp", bufs=1) as y2stp,
            tc.tile_pool(name="outp", bufs=3) as outp,
            tc.tile_pool(name="ps1", bufs=2, space="PSUM") as ps1,
            tc.tile_pool(name="pst", bufs=2, space="PSUM") as pst,
            tc.tile_pool(name="ps3", bufs=1, space="PSUM") as ps3,
        ):
            ident = constp.tile([P, P], dt.float16)
            make_identity(nc, ident[:])

            x_sb = xp.tile([P, KT1 * T], dt.float32r)
            nc.sync.dma_start(
                out=x_sb[:].rearrange("p (kt t) -> p kt t", kt=KT1),
                in_=xT[:].rearrange("(kt p) t -> p kt t", p=P),
            )
            y2sT = y2stp.tile([P, KT2 * T], dt.float16)

            # ---- phase 1: y1 = x @ w1T, squared-relu, 2:4 sparsify, transpose
            G = CH1 // 4
            for n in range(NI):
                w1_sb = w1p.tile([P, KT1 * CH1], dt.float32r, tag="w1")
                nc.sync.dma_start(
                    out=w1_sb[:].rearrange("p (kt i) -> p kt i", kt=KT1),
                    in_=w1T[:, n * CH1:(n + 1) * CH1].rearrange(
                        "(kt p) i -> p kt i", p=P
                    ),
                )
                for m in range(MT):
                    acc = ps1.tile([P, CH1], dt.float32, tag="ps1")
                    for kt in range(KT1):
                        nc.tensor.matmul(
                            acc[:],
                            lhsT=x_sb[:, kt * T + m * P: kt * T + (m + 1) * P],
                            rhs=w1_sb[:, kt * CH1:(kt + 1) * CH1],
                            start=(kt == 0),
                            stop=(kt == KT1 - 1),
                        )
                    y2r = actp.tile([P, CH1], dt.float32, tag="y2r")
                    nc.vector.tensor_scalar_max(y2r[:], acc[:], 0.0)
                    # threshold = 2nd largest of each group of 4 (on relu out)
                    pr = y2r[:].rearrange("p (g two) -> p g two", two=2)
                    mx = actp.tile([P, CH1 // 2], dt.float32, tag="mx")
                    mn = actp.tile([P, CH1 // 2], dt.float32, tag="mn")
                    nc.vector.tensor_tensor(
                        mx[:].rearrange("p (g one) -> p g one", one=1),
                        pr[:, :, 0:1], pr[:, :, 1:2], ALU.max)
                    nc.vector.tensor_tensor(
                        mn[:].rearrange("p (g one) -> p g one", one=1),
                        pr[:, :, 0:1], pr[:, :, 1:2], ALU.min)
                    mxp = mx[:].rearrange("p (g two) -> p g two", two=2)
                    mnp = mn[:].rearrange("p (g two) -> p g two", two=2)
                    a = actp.tile([P, G], dt.float32, tag="a")
                    b = actp.tile([P, G], dt.float32, tag="b")
                    thr = actp.tile([P, G], dt.float32, tag="thr")
                    nc.vector.tensor_tensor(
                        a[:].rearrange("p (g one) -> p g one", one=1),
                        mxp[:, :, 0:1], mxp[:, :, 1:2], ALU.min)
                    nc.vector.tensor_tensor(
                        b[:].rearrange("p (g one) -> p g one", one=1),
                        mnp[:, :, 0:1], mnp[:, :, 1:2], ALU.max)
                    nc.vector.tensor_tensor(thr[:], a[:], b[:], ALU.max)
                    # keep = y2r >= thr (ties at 0 keep extra zeros: harmless)
                    ge = actp.tile([P, CH1], dt.float32, tag="ge")
                    thr_b = thr[:].rearrange(
                        "p (g one) -> p g one", one=1).to_broadcast([P, G, 4])
                    nc.vector.tensor_tensor(
                        ge[:].rearrange("p (g four) -> p g four", four=4),
                        y2r[:].rearrange("p (g four) -> p g four", four=4),
                        thr_b, ALU.is_ge)
                    ym = actp.tile([P, CH1], dt.float32, tag="ym")
                    nc.vector.tensor_tensor(ym[:], ge[:], y2r[:], ALU.mult)
                    y2s = actp.tile([P, CH1], dt.float16, tag="y2s")
                    nc.vector.tensor_tensor(y2s[:], ym[:], ym[:], ALU.mult)
                    # transpose [tok, i] -> [i, tok] via PE
                    ptt = pst.tile([P, CH1], dt.float16, tag="pst", space="PSUM")
                    for j in range(KPC):
                        nc.tensor.transpose(
                            ptt[:, j * P:(j + 1) * P],
                            y2s[:, j * P:(j + 1) * P], ident[:])
                    dst = y2sT[:].rearrange("p (kt t) -> p kt t", kt=KT2)[
                        :, n * KPC:(n + 1) * KPC, m * P:(m + 1) * P]
                    nc.scalar.copy(
                        out=dst, in_=ptt[:].rearrange("p (j t) -> p j t", j=KPC))

            # ---- phase 2: y3 = y2s @ w2T, accumulated over all 64 i k-tiles
            for c in range(NH):
                accs = [ps3.tile([P, 512], dt.float32, tag=f"ps3_{m}",
                                 name=f"acc3_{c}_{m}")
                        for m in range(MT)]
                for q in range(4):
                    w2_sb = w2p.tile([P, 16 * 512], dt.float16, tag="w2")
                    nc.sync.dma_start(
                        out=w2_sb[:].rearrange("p (kt h) -> p kt h", kt=16),
                        in_=w2T[q * 16 * P:(q + 1) * 16 * P,
                                c * 512:(c + 1) * 512].rearrange(
                            "(kt p) h -> p kt h", p=P),
                    )
                    for m in range(MT):
                        for kt in range(16):
                            kt2 = q * 16 + kt
                            nc.tensor.matmul(
                                accs[m][:],
                                lhsT=y2sT[:, kt2 * T + m * P: kt2 * T + (m + 1) * P],
                                rhs=w2_sb[:, kt * 512:(kt + 1) * 512],
                                start=(kt2 == 0),
                                stop=(kt2 == KT2 - 1),
                            )
                for m in range(MT):
                    o_sb = outp.tile([P, 512], dt.float32, tag="o")
                    nc.scalar.copy(out=o_sb[:], in_=accs[m][:])
                    nc.sync.dma_start(
                        out=y3[m * P:(m + 1) * P, c * 512:(c + 1) * 512],
                        in_=o_sb[:])
    nc.finalize()
    return nc


def _get_built():
    global _built
    if _built is None:
        _built = _build()
    return _built


def _prep_in_maps(x, w1, w2, perm):
    xp_full = x[:, perm, :].reshape(B * S, H)
    w1T = np.ascontiguousarray(w1.T)                     # [H, I] f32
    w2T = np.ascontiguousarray(w2.T).astype(np.float16)  # [I, H] bf16
    in_maps = []
    for k in range(NCORES):
        xT_k = np.ascontiguousarray(xp_full[k * T:(k + 1) * T].T)  # [H, T]
        in_maps.append({"xT": xT_k, "w1T": w1T, "w2T": w2T})
    return in_maps


def run(x, w1, w2, perm, trace=False):
    nc = _get_built()
    in_maps = _prep_in_maps(x, w1, w2, perm)
    res = run_bass_kernel_spmd(nc, in_maps, core_ids=list(range(NCORES)),
                               trace=trace)
    y3_full = np.concatenate([res.results[k]["y3"] for k in range(NCORES)],
                             axis=0)  # [B*S, H] in permuted order
    out = np.empty((B, S, H), dtype=np.float32)
    out[:, perm, :] = y3_full.reshape(B, S, H)
    return out, res


def kernel(x, w1, w2, perm):
    out, _ = run(np.asarray(x, dtype=np.float32),
                 np.asarray(w1, dtype=np.float32),
                 np.asarray(w2, dtype=np.float32),
                 np.asarray(perm, dtype=np.int32))
    return out

